# revision 33
# baseline (speedup 1.0000x reference)
import numpy as np

import concourse.bacc as bacc
import concourse.bass as bass
import concourse.mybir as mybir
import concourse.tile as tile
from concourse.bass_utils import run_bass_kernel_spmd

# HDRNet color model, fully on-device. 8 cores, data-parallel over
# (batch, row-strip): core k handles image k//4, full-res rows
# [(k%4)*256, (k%4)*256+256).
#
# Per core:
#   1. Banded CNN: each core computes only the 6 bilateral-grid rows its
#      strip samples. Grid row g needs conv6 rows {16g+7,16g+8}, whose
#      receptive field is xlow rows [16g+1, 16g+15) — host pre-slices those
#      14-row bands so the kernel is uniform across cores.
#   2. Grid -> per-pixel coefficients via two interpolation matmuls
#      (row-interp onto the strip's 256 rows, col-interp onto 1024 cols),
#      one [128,1024] plane per (coeff, depth) channel.
#   3. Depth lerp as a dense 8-tent weighted sum (exact for d in [0,7]),
#      then the per-pixel 3x4 affine apply + clip.
# Host only does the 4x4 bilinear downsample, band slicing, and weight
# repacking (all cheap numpy).

B, C, H, W = 2, 3, 1024, 1024
HG, WG, DG, NP = 16, 16, 8, 12
N_CORES = 8
STRIP = H // 4
GN = 6                     # grid rows computed per core
GBASE = [0, 3, 7, 10]      # first grid row per strip quarter
LAYERS = [(3, 16), (16, 32), (32, 32), (32, 64), (64, 64), (64, 96)]
F32 = mybir.dt.float32

_CACHED = {}


# K-packed conv weight shapes per layer: L0 packs all 9 taps (K=27);
# L1-3 pack the 3 dx taps per dy (K=3*Cin); L4-5 pack dx pairs (K=128)
# plus a K=64 single-dx group.
def _wshapes(i, ci, co):
    if i == 0:
        return [(27, 1, co)]
    if ci <= 32:
        return [(3 * ci, 3, co)]
    return [(128, 3, co), (64, 3, co)]


# packed input layout: [xs | bands | wyT | wxm | weight groups | biases]
_OFFS = {}
_off = 0
for _name, _sz in [("xs", C * STRIP * W), ("bands", GN * C * 14 * 256),
                   ("wyT", GN * STRIP), ("wxm", WG * W)]:
    _OFFS[_name] = _off
    _off += _sz
for _i, (_ci, _co) in enumerate(LAYERS):
    for _g, (_k, _d, _o) in enumerate(_wshapes(_i, _ci, _co)):
        _OFFS[f"cw{_i}g{_g}"] = _off
        _off += _k * _d * _o
for _i, (_ci, _co) in enumerate(LAYERS):
    _OFFS[f"cb{_i}"] = _off
    _off += _co
NTOT = _off


def _build_module():
    nc = bacc.Bacc("TRN2", target_bir_lowering=False, debug=False,
                   num_devices=N_CORES)
    inp_t = nc.dram_tensor("inp", [NTOT], F32, kind="ExternalInput")
    ys_t = nc.dram_tensor("ys", [C, STRIP, W], mybir.dt.uint16,
                          kind="ExternalOutput")

    inp = inp_t.ap()

    def sub(name, size, pattern, **kw):
        return inp[_OFFS[name]: _OFFS[name] + size].rearrange(pattern, **kw)

    xs = sub("xs", C * STRIP * W, "(c h w) -> c h w", c=C, h=STRIP, w=W)
    bands = sub("bands", GN * C * 14 * 256, "(j c r w) -> j c r w",
                j=GN, c=C, r=14, w=256)
    wyT = sub("wyT", GN * STRIP, "(g r) -> g r", g=GN, r=STRIP)
    wxm = sub("wxm", WG * W, "(g w) -> g w", g=WG, w=W)
    cw = [[sub(f"cw{i}g{g}", k * dd * o, "(a t o) -> a t o", a=k, t=dd, o=o)
           for g, (k, dd, o) in enumerate(_wshapes(i, ci, co))]
          for i, (ci, co) in enumerate(LAYERS)]
    cb = [sub(f"cb{i}", co, "(o u) -> o u", o=co, u=1)
          for i, (ci, co) in enumerate(LAYERS)]
    ys = ys_t.ap()

    mm = nc.tensor.matmul
    Alu = mybir.AluOpType
    Act = mybir.ActivationFunctionType

    with tile.TileContext(nc) as tc:
        with tc.tile_pool(name="const", bufs=1) as constp:
            # ---- constants
            wxS = constp.tile([WG, W], F32, tag="wx")
            nc.sync.dma_start(wxS[:], wxm[:, :])
            wyS = constp.tile([GN, STRIP], F32, tag="wy")
            nc.sync.dma_start(wyS[:], wyT[:, :])
            cwS, cbS = [], []
            for i, (ci, co) in enumerate(LAYERS):
                grps = []
                for g, (k, dd, o_) in enumerate(_wshapes(i, ci, co)):
                    t = constp.tile([k, dd, o_], F32, tag=f"cw{i}g{g}")
                    nc.sync.dma_start(t[:, :, :], cw[i][g][:, :, :])
                    grps.append(t)
                cwS.append(grps)
                tb = constp.tile([co, 1], F32, tag=f"cb{i}")
                nc.sync.dma_start(tb[:], cb[i][:, :])
                cbS.append(tb)
            gridSB = constp.tile([96, GN, WG], F32, tag="grid")
            gridT = constp.tile([GN, 96, WG], F32, tag="gridT")

            # ---- banded CNN -> gridSB [96, GN, 16]
            # Conv taps are K-packed: dx-shifted replicas of each layer's
            # activation are stacked along partitions so one matmul covers
            # several taps.
            with (
                tc.tile_pool(name="act", bufs=1) as actp,
                tc.tile_pool(name="repl", bufs=1) as replp,
                tc.tile_pool(name="cps", bufs=1, space="PSUM") as cpsp,
                tc.tile_pool(name="c6ps", bufs=1, space="PSUM") as c6psp,
                tc.tile_pool(name="gtmp", bufs=4) as gtmpp,
            ):
                for j in range(GN):
                    a = actp.tile([C, 14, 258], F32, tag="l0")
                    nc.vector.memset(a[:, :, 0:258:257], 0.0)
                    nc.sync.dma_start(a[:, :, 1:257], bands[j, :, :, :])
                    rows = 14
                    for li, (ci, co) in enumerate(LAYERS):
                        rows_out = rows - 2
                        last = li == 5
                        wg = cwS[li]
                        # build dx-shifted replicas of `a`
                        if li == 0:
                            rp1 = replp.tile([9, rows, 258], F32, tag="rp0a")
                            for t1 in range(3):
                                nc.sync.dma_start(
                                    rp1[t1 * ci: (t1 + 1) * ci, :, 0:258 - t1],
                                    a[:, :, t1:258])
                            # second level: dy shifts -> K=27
                            rp = replp.tile([27, rows, 258], F32, tag="rp0b")
                            for t2 in range(3):
                                nc.sync.dma_start(
                                    rp[t2 * 9: (t2 + 1) * 9, 0: rows - t2, :],
                                    rp1[:, t2: rows, :])
                            nrep = 3
                        elif ci <= 32:
                            rp = replp.tile([3 * ci, rows, 258], F32,
                                            tag=f"rp{li}")
                            for t1 in range(3):
                                nc.sync.dma_start(
                                    rp[t1 * ci: (t1 + 1) * ci, :, 0:258 - t1],
                                    a[:, :, t1:258])
                            nrep = 3
                        else:
                            rp = replp.tile([2 * ci, rows, 258], F32,
                                            tag=f"rp{li}")
                            for t1 in range(2):
                                nc.sync.dma_start(
                                    rp[t1 * ci: (t1 + 1) * ci, :, 0:258 - t1],
                                    a[:, :, t1:258])
                            nrep = 2
                        pool = c6psp if last else cpsp
                        ps3 = pool.tile([co, rows_out, 256], F32,
                                        tag="c6" if last else "cps")
                        for r0 in range(0, rows_out, 2):
                            rc = min(2, rows_out - r0)
                            out_ap = ps3[:, r0: r0 + rc, :]
                            if li == 0:
                                mm(out_ap, wg[0][:, 0, :],
                                   rp[:, r0: r0 + rc, 0:256],
                                   start=True, stop=True)
                            elif ci <= 32:
                                for dy in range(3):
                                    mm(out_ap, wg[0][:, dy, :],
                                       rp[:, dy + r0: dy + r0 + rc, 0:256],
                                       start=(dy == 0), stop=(dy == 2))
                            else:
                                for dy in range(3):
                                    mm(out_ap, wg[0][:, dy, :],
                                       rp[:, dy + r0: dy + r0 + rc, 0:256],
                                       start=(dy == 0), stop=False)
                                    mm(out_ap, wg[1][:, dy, :],
                                       rp[0:ci, dy + r0: dy + r0 + rc, 2:258],
                                       start=False, stop=(dy == 2))
                        if not last:
                            o = actp.tile([co, rows_out, 258], F32,
                                          tag=f"l{li + 1}")
                            nc.vector.memset(o[:, :, 0:258:257], 0.0)
                            nc.scalar.activation(
                                o[:, :, 1:257], ps3[:, :, :],
                                Act.Relu, bias=cbS[li][:, :], scale=1.0)
                        if last:
                            # evacuate conv6 + bias, then avg rows+cols
                            c6sb = gtmpp.tile([96, 2, 256], F32, tag="c6sb")
                            nc.scalar.activation(
                                c6sb[:, :, :], ps3[:, :, :], Act.Identity,
                                bias=cbS[5][:, :], scale=1.0)
                            ta = gtmpp.tile([96, WG], F32, tag="ga")
                            tb = gtmpp.tile([96, WG], F32, tag="gb")
                            nc.vector.tensor_tensor(
                                ta[:], c6sb[:, 0, 7:256:16],
                                c6sb[:, 1, 7:256:16], Alu.add)
                            nc.vector.tensor_tensor(
                                tb[:], c6sb[:, 0, 8:256:16],
                                c6sb[:, 1, 8:256:16], Alu.add)
                            nc.vector.tensor_tensor(ta[:], ta[:], tb[:],
                                                    Alu.add)
                            nc.vector.tensor_scalar(
                                gridSB[:, j, :], ta[:], 0.25, None,
                                op0=Alu.mult)
                        else:
                            a = o
                            rows = rows_out
                # rearrange grid [96ch, GN, 16] -> [GN, ch, 16]
                for j in range(GN):
                    nc.sync.dma_start(gridT[j: j + 1, :, :],
                                      gridSB[:, j, :])

            # ---- full-res slice + affine, per 128-row block
            with (
                tc.tile_pool(name="xsp", bufs=3) as xsp,
                tc.tile_pool(name="dg", bufs=2) as dgp,
                tc.tile_pool(name="tent", bufs=8) as tentp,
                tc.tile_pool(name="outp", bufs=3) as outp,
                tc.tile_pool(name="acc", bufs=12) as accp,
                tc.tile_pool(name="mtmpa", bufs=4) as mtmpp,
                tc.tile_pool(name="yg", bufs=3) as ygp,
                tc.tile_pool(name="ygps", bufs=2, space="PSUM") as ygpsp,
                tc.tile_pool(name="gps", bufs=1, space="PSUM") as gpsp,
            ):
                for blk in range(2):
                    r0 = blk * 128
                    xt = []
                    for ch in range(C):
                        t = xsp.tile([128, W], F32, tag="x")
                        nc.sync.dma_start(t[:], xs[ch, r0: r0 + 128, :])
                        xt.append(t)
                    # guide -> d in [0,7]
                    lum = dgp.tile([128, W], F32, tag="lum")
                    t2 = dgp.tile([128, W], F32, tag="lt")
                    nc.vector.tensor_scalar(lum[:], xt[0][:], 0.299, None,
                                            op0=Alu.mult)
                    nc.vector.tensor_scalar(t2[:], xt[1][:], 0.587, None,
                                            op0=Alu.mult)
                    nc.vector.tensor_tensor(lum[:], lum[:], t2[:], Alu.add)
                    nc.vector.tensor_scalar(t2[:], xt[2][:], 0.114, None,
                                            op0=Alu.mult)
                    nc.vector.tensor_tensor(lum[:], lum[:], t2[:], Alu.add)
                    nc.vector.tensor_scalar(lum[:], lum[:], 0.0, 1.0,
                                            op0=Alu.max, op1=Alu.min)
                    nc.vector.tensor_scalar(lum[:], lum[:], 7.0, None,
                                            op0=Alu.mult)
                    tents = []
                    for k in range(8):
                        # w_k = relu(1-|d-k|) = relu(min(d-(k-1), (k+1)-d))
                        u = tentp.tile([128, W], F32, tag="tent")
                        ub = dgp.tile([128, W], F32, tag="tb")
                        nc.vector.tensor_scalar(u[:], lum[:], float(k - 1),
                                                None, op0=Alu.subtract)
                        nc.vector.tensor_scalar(ub[:], lum[:], -1.0,
                                                float(k + 1), op0=Alu.mult,
                                                op1=Alu.add)
                        nc.vector.tensor_tensor(u[:], u[:], ub[:], Alu.min)
                        nc.vector.tensor_scalar(u[:], u[:], 0.0, None,
                                                op0=Alu.max)
                        tents.append(u)

                    accs = []
                    for cgrp in range(12):
                        # stage-Y: 8 channels (c=cgrp, k=0..7)
                        ygps = ygpsp.tile([WG, 8, 128], F32, tag="ygps")
                        for k in range(8):
                            mm(ygps[:, k, :], gridT[:, cgrp * 8 + k, :],
                               wyS[:, r0: r0 + 128], start=True, stop=True)
                        yg = ygp.tile([WG, 8, 128], F32, tag="yg")
                        nc.scalar.copy(yg[:, :, :], ygps[:, :, :])
                        acc = accp.tile([128, W], F32, tag="acc")
                        accs.append(acc)
                        for kp in range(4):
                            # two depth-planes share one PSUM tile: halves
                            # the PE->DVE handoff count
                            gp2 = gpsp.tile([128, 2, W], F32, tag="gps")
                            for kh in range(2):
                                k = 2 * kp + kh
                                lhsT = yg[:, k, :]
                                mm(gp2[:, kh, 0:512], lhsT, wxS[:, 0:512],
                                   start=True, stop=True)
                                mm(gp2[:, kh, 512:1024], lhsT,
                                   wxS[:, 512:1024], start=True, stop=True)
                            for kh in range(2):
                                k = 2 * kp + kh
                                if k == 0:
                                    nc.vector.tensor_tensor(
                                        acc[:], tents[0][:], gp2[:, 0, :],
                                        Alu.mult)
                                else:
                                    tm = mtmpp.tile([128, W], F32, tag="mt")
                                    nc.vector.tensor_tensor(
                                        tm[:], tents[k][:], gp2[:, kh, :],
                                        Alu.mult)
                                    nc.vector.tensor_tensor(
                                        acc[:], acc[:], tm[:], Alu.add)
                    # affine apply + clip
                    for i in range(3):
                        o = accs[4 * i + 3]
                        for jj in range(3):
                            tm = mtmpp.tile([128, W], F32, tag="at")
                            nc.vector.tensor_tensor(
                                tm[:], accs[4 * i + jj][:], xt[jj][:],
                                Alu.mult)
                            nc.vector.tensor_tensor(o[:], o[:], tm[:],
                                                    Alu.add)
                        nc.vector.tensor_scalar(o[:], o[:], 0.0, 1.0,
                                                op0=Alu.max, op1=Alu.min)
                        # quantize to uint16 (DVE convert rounds to nearest)
                        ot = outp.tile([128, W], mybir.dt.uint16, tag="out")
                        nc.vector.tensor_scalar(ot[:], o[:], 65535.0, None,
                                                op0=Alu.mult)
                        nc.sync.dma_start(ys[i, r0: r0 + 128, :], ot[:])
    nc.compile()
    return nc


def _host_prep(x, ws):
    x = np.ascontiguousarray(x, np.float32)
    t = 0.5 * (x[:, :, 1::4, :] + x[:, :, 2::4, :])
    xl = 0.5 * (t[:, :, :, 1::4] + t[:, :, :, 2::4])  # [B,3,256,256]

    bands = np.empty((N_CORES, GN, C, 14, 256), np.float32)
    for k in range(N_CORES):
        b, q = k // 4, k % 4
        for j in range(GN):
            g = GBASE[q] + j
            bands[k, j] = xl[b, :, 16 * g + 1: 16 * g + 15, :]

    if "const" in _CACHED:
        wyT, wxm = _CACHED["const"]
        cws, cbs = _repack_weights(ws)
        return x, bands, wyT, wxm, cws, cbs

    ysf = np.arange(H, dtype=np.float32) * np.float32((HG - 1) / (H - 1))
    y0 = np.floor(ysf).astype(np.int32)
    wy = (ysf - y0).astype(np.float32)
    wyT = np.zeros((N_CORES, GN, STRIP), np.float32)
    for k in range(N_CORES):
        q = k % 4
        rs = q * STRIP
        for r in range(STRIP):
            rr = rs + r
            j0 = y0[rr] - GBASE[q]
            j1 = min(y0[rr] + 1, HG - 1) - GBASE[q]
            wyT[k, j0, r] += np.float32(1.0) - wy[rr]
            wyT[k, j1, r] += wy[rr]

    xsf = np.arange(W, dtype=np.float32) * np.float32((WG - 1) / (W - 1))
    x0 = np.floor(xsf).astype(np.int32)
    wx = (xsf - x0).astype(np.float32)
    wxm = np.zeros((WG, W), np.float32)
    for ccol in range(W):
        x1 = min(x0[ccol] + 1, WG - 1)
        wxm[x0[ccol], ccol] += np.float32(1.0) - wx[ccol]
        wxm[x1, ccol] += wx[ccol]

    _CACHED["const"] = (wyT, wxm)
    cws, cbs = _repack_weights(ws)
    return x, bands, wyT, wxm, cws, cbs


def _repack_weights(ws):
    cws, cbs = [], []
    for i in range(6):
        wl = np.asarray(ws[2 * i], np.float32)  # [O, I, 3, 3]
        O, I = wl.shape[0], wl.shape[1]
        if i == 0:
            # [27, 1, O]: q = dy*9 + dx*3 + i
            g0 = wl.transpose(2, 3, 1, 0).reshape(27, 1, O)
            cws.append([np.ascontiguousarray(g0)])
        elif I <= 32:
            # [3I, 3, O]: per dy, q = dx*I + i
            g0 = np.stack([wl[:, :, dy, :].transpose(2, 1, 0).reshape(3 * I, O)
                           for dy in range(3)], axis=1)
            cws.append([np.ascontiguousarray(g0)])
        else:
            g0 = np.stack(
                [wl[:, :, dy, 0:2].transpose(2, 1, 0).reshape(2 * I, O)
                 for dy in range(3)], axis=1)
            g1 = np.stack([wl[:, :, dy, 2].transpose(1, 0)
                           for dy in range(3)], axis=1)
            cws.append([np.ascontiguousarray(g0), np.ascontiguousarray(g1)])
        cbs.append(np.ascontiguousarray(
            np.asarray(ws[2 * i + 1], np.float32).reshape(-1, 1)))
    return cws, cbs


def kernel(x, w1, b1, w2, b2, w3, b3, w4, b4, w5, b5, w6, b6):
    ws = (w1, b1, w2, b2, w3, b3, w4, b4, w5, b5, w6, b6)
    x, bands, wyT, wxm, cws, cbs = _host_prep(x, ws)
    if "nc" not in _CACHED:
        _CACHED["nc"] = _build_module()
    nc = _CACHED["nc"]

    in_maps = []
    for k in range(N_CORES):
        b, s = k // 4, (k % 4) * STRIP
        blob = np.empty(NTOT, np.float32)
        blob[_OFFS["xs"]: _OFFS["xs"] + C * STRIP * W] = \
            x[b, :, s: s + STRIP, :].reshape(-1)
        blob[_OFFS["bands"]: _OFFS["bands"] + bands[k].size] = \
            bands[k].reshape(-1)
        blob[_OFFS["wyT"]: _OFFS["wyT"] + wyT[k].size] = wyT[k].reshape(-1)
        blob[_OFFS["wxm"]: _OFFS["wxm"] + wxm.size] = wxm.reshape(-1)
        for i in range(6):
            for g, arr in enumerate(cws[i]):
                o = _OFFS[f"cw{i}g{g}"]
                blob[o: o + arr.size] = arr.reshape(-1)
            o = _OFFS[f"cb{i}"]
            blob[o: o + cbs[i].size] = cbs[i].reshape(-1)
        in_maps.append({"inp": blob})
    res = run_bass_kernel_spmd(nc, in_maps, core_ids=list(range(N_CORES)))
    _CACHED["last"] = res
    y = np.empty((B, C, H, W), np.float32)
    scale = np.float32(1.0 / 65535.0)
    for k in range(N_CORES):
        b, s = k // 4, (k % 4) * STRIP
        q = res.results[k]["ys"]
        np.multiply(q, scale, out=y[b, :, s: s + STRIP, :],
                    casting="unsafe")
    return y


# revision 34
# speedup vs baseline: 1.0578x; 1.0578x over previous
import numpy as np

import concourse.bacc as bacc
import concourse.bass as bass
import concourse.mybir as mybir
import concourse.tile as tile
from concourse.bass_utils import run_bass_kernel_spmd

# HDRNet color model, fully on-device. 8 cores, data-parallel over
# (batch, row-strip): core k handles image k//4, full-res rows
# [(k%4)*256, (k%4)*256+256).
#
# Per core:
#   1. Banded CNN: each core computes only the 6 bilateral-grid rows its
#      strip samples. Grid row g needs conv6 rows {16g+7,16g+8}, whose
#      receptive field is xlow rows [16g+1, 16g+15) — host pre-slices those
#      14-row bands so the kernel is uniform across cores.
#   2. Grid -> per-pixel coefficients via two interpolation matmuls
#      (row-interp onto the strip's 256 rows, col-interp onto 1024 cols),
#      one [128,1024] plane per (coeff, depth) channel.
#   3. Depth lerp as a dense 8-tent weighted sum (exact for d in [0,7]),
#      then the per-pixel 3x4 affine apply + clip.
# Host only does the 4x4 bilinear downsample, band slicing, and weight
# repacking (all cheap numpy).

B, C, H, W = 2, 3, 1024, 1024
HG, WG, DG, NP = 16, 16, 8, 12
N_CORES = 8
STRIP = H // 4
GN = 6                     # grid rows computed per core
GBASE = [0, 3, 7, 10]      # first grid row per strip quarter
LAYERS = [(3, 16), (16, 32), (32, 32), (32, 64), (64, 64), (64, 96)]
F32 = mybir.dt.float32

_CACHED = {}


# K-packed conv weight shapes per layer: L0 packs all 9 taps (K=27);
# L1-3 pack the 3 dx taps per dy (K=3*Cin); L4-5 pack dx pairs (K=128)
# plus a K=64 single-dx group.
def _wshapes(i, ci, co):
    if i == 0:
        return [(27, 1, co)]
    if ci <= 32:
        return [(3 * ci, 3, co)]
    return [(128, 3, co), (64, 3, co)]


# packed input layout: [xs | bands | wyT | wxm | weight groups | biases]
_OFFS = {}
_off = 0
for _name, _sz in [("xs", C * STRIP * W), ("bands", GN * C * 14 * 256),
                   ("wyT", GN * STRIP), ("wxm", WG * W)]:
    _OFFS[_name] = _off
    _off += _sz
for _i, (_ci, _co) in enumerate(LAYERS):
    for _g, (_k, _d, _o) in enumerate(_wshapes(_i, _ci, _co)):
        _OFFS[f"cw{_i}g{_g}"] = _off
        _off += _k * _d * _o
for _i, (_ci, _co) in enumerate(LAYERS):
    _OFFS[f"cb{_i}"] = _off
    _off += _co
NTOT = _off


def _build_module():
    nc = bacc.Bacc("TRN2", target_bir_lowering=False, debug=False,
                   num_devices=N_CORES)
    inp_t = nc.dram_tensor("inp", [NTOT], F32, kind="ExternalInput")
    ys_t = nc.dram_tensor("ys", [C, STRIP, W], mybir.dt.uint16,
                          kind="ExternalOutput")

    inp = inp_t.ap()

    def sub(name, size, pattern, **kw):
        return inp[_OFFS[name]: _OFFS[name] + size].rearrange(pattern, **kw)

    xs = sub("xs", C * STRIP * W, "(c h w) -> c h w", c=C, h=STRIP, w=W)
    bands = sub("bands", GN * C * 14 * 256, "(j c r w) -> j c r w",
                j=GN, c=C, r=14, w=256)
    wyT = sub("wyT", GN * STRIP, "(g r) -> g r", g=GN, r=STRIP)
    wxm = sub("wxm", WG * W, "(g w) -> g w", g=WG, w=W)
    cw = [[sub(f"cw{i}g{g}", k * dd * o, "(a t o) -> a t o", a=k, t=dd, o=o)
           for g, (k, dd, o) in enumerate(_wshapes(i, ci, co))]
          for i, (ci, co) in enumerate(LAYERS)]
    cb = [sub(f"cb{i}", co, "(o u) -> o u", o=co, u=1)
          for i, (ci, co) in enumerate(LAYERS)]
    ys = ys_t.ap()

    mm = nc.tensor.matmul
    Alu = mybir.AluOpType
    Act = mybir.ActivationFunctionType

    with tile.TileContext(nc) as tc:
        with tc.tile_pool(name="const", bufs=1) as constp:
            # ---- constants
            wxS = constp.tile([WG, W], F32, tag="wx")
            nc.sync.dma_start(wxS[:], wxm[:, :])
            wyS = constp.tile([GN, STRIP], F32, tag="wy")
            nc.sync.dma_start(wyS[:], wyT[:, :])
            cwS, cbS = [], []
            for i, (ci, co) in enumerate(LAYERS):
                grps = []
                for g, (k, dd, o_) in enumerate(_wshapes(i, ci, co)):
                    t = constp.tile([k, dd, o_], F32, tag=f"cw{i}g{g}")
                    nc.sync.dma_start(t[:, :, :], cw[i][g][:, :, :])
                    grps.append(t)
                cwS.append(grps)
                tb = constp.tile([co, 1], F32, tag=f"cb{i}")
                nc.sync.dma_start(tb[:], cb[i][:, :])
                cbS.append(tb)
            gridSB = constp.tile([96, GN, WG], F32, tag="grid")
            gridT = constp.tile([GN, 96, WG], F32, tag="gridT")

            # ---- banded CNN -> gridSB [96, GN, 16]
            # Conv taps are K-packed: dx-shifted replicas of each layer's
            # activation are stacked along partitions so one matmul covers
            # several taps.
            with (
                tc.tile_pool(name="act", bufs=1) as actp,
                tc.tile_pool(name="repl", bufs=1) as replp,
                tc.tile_pool(name="cps", bufs=1, space="PSUM") as cpsp,
                tc.tile_pool(name="c6ps", bufs=1, space="PSUM") as c6psp,
                tc.tile_pool(name="gtmp", bufs=4) as gtmpp,
            ):
                for j in range(GN):
                    a = actp.tile([C, 14, 258], F32, tag="l0")
                    nc.vector.memset(a[:, :, 0:258:257], 0.0)
                    nc.sync.dma_start(a[:, :, 1:257], bands[j, :, :, :])
                    rows = 14
                    for li, (ci, co) in enumerate(LAYERS):
                        rows_out = rows - 2
                        last = li == 5
                        wg = cwS[li]
                        # build dx-shifted replicas of `a`
                        if li == 0:
                            rp1 = replp.tile([9, rows, 258], F32, tag="rp0a")
                            for t1 in range(3):
                                nc.sync.dma_start(
                                    rp1[t1 * ci: (t1 + 1) * ci, :, 0:258 - t1],
                                    a[:, :, t1:258])
                            # second level: dy shifts -> K=27
                            rp = replp.tile([27, rows, 258], F32, tag="rp0b")
                            for t2 in range(3):
                                nc.sync.dma_start(
                                    rp[t2 * 9: (t2 + 1) * 9, 0: rows - t2, :],
                                    rp1[:, t2: rows, :])
                            nrep = 3
                        elif ci <= 32:
                            rp = replp.tile([3 * ci, rows, 258], F32,
                                            tag=f"rp{li}")
                            for t1 in range(3):
                                nc.sync.dma_start(
                                    rp[t1 * ci: (t1 + 1) * ci, :, 0:258 - t1],
                                    a[:, :, t1:258])
                            nrep = 3
                        else:
                            rp = replp.tile([2 * ci, rows, 258], F32,
                                            tag=f"rp{li}")
                            for t1 in range(2):
                                nc.sync.dma_start(
                                    rp[t1 * ci: (t1 + 1) * ci, :, 0:258 - t1],
                                    a[:, :, t1:258])
                            nrep = 2
                        pool = c6psp if last else cpsp
                        ps3 = pool.tile([co, rows_out, 256], F32,
                                        tag="c6" if last else "cps")
                        for r0 in range(0, rows_out, 2):
                            rc = min(2, rows_out - r0)
                            out_ap = ps3[:, r0: r0 + rc, :]
                            if li == 0:
                                mm(out_ap, wg[0][:, 0, :],
                                   rp[:, r0: r0 + rc, 0:256],
                                   start=True, stop=True)
                            elif ci <= 32:
                                for dy in range(3):
                                    mm(out_ap, wg[0][:, dy, :],
                                       rp[:, dy + r0: dy + r0 + rc, 0:256],
                                       start=(dy == 0), stop=(dy == 2))
                            else:
                                for dy in range(3):
                                    mm(out_ap, wg[0][:, dy, :],
                                       rp[:, dy + r0: dy + r0 + rc, 0:256],
                                       start=(dy == 0), stop=False)
                                    mm(out_ap, wg[1][:, dy, :],
                                       rp[0:ci, dy + r0: dy + r0 + rc, 2:258],
                                       start=False, stop=(dy == 2))
                        if not last:
                            o = actp.tile([co, rows_out, 258], F32,
                                          tag=f"l{li + 1}")
                            nc.vector.memset(o[:, :, 0:258:257], 0.0)
                            nc.scalar.activation(
                                o[:, :, 1:257], ps3[:, :, :],
                                Act.Relu, bias=cbS[li][:, :], scale=1.0)
                        if last:
                            # evacuate conv6 + bias, then avg rows+cols
                            c6sb = gtmpp.tile([96, 2, 256], F32, tag="c6sb")
                            nc.scalar.activation(
                                c6sb[:, :, :], ps3[:, :, :], Act.Identity,
                                bias=cbS[5][:, :], scale=1.0)
                            ta = gtmpp.tile([96, WG], F32, tag="ga")
                            tb = gtmpp.tile([96, WG], F32, tag="gb")
                            nc.vector.tensor_tensor(
                                ta[:], c6sb[:, 0, 7:256:16],
                                c6sb[:, 1, 7:256:16], Alu.add)
                            nc.vector.tensor_tensor(
                                tb[:], c6sb[:, 0, 8:256:16],
                                c6sb[:, 1, 8:256:16], Alu.add)
                            nc.vector.tensor_tensor(ta[:], ta[:], tb[:],
                                                    Alu.add)
                            nc.vector.tensor_scalar(
                                gridSB[:, j, :], ta[:], 0.25, None,
                                op0=Alu.mult)
                        else:
                            a = o
                            rows = rows_out
                # rearrange grid [96ch, GN, 16] -> [GN, ch, 16]
                for j in range(GN):
                    nc.sync.dma_start(gridT[j: j + 1, :, :],
                                      gridSB[:, j, :])

            # ---- full-res slice + affine, per 128-row block
            with (
                tc.tile_pool(name="xsp", bufs=3) as xsp,
                tc.tile_pool(name="dg", bufs=2) as dgp,
                tc.tile_pool(name="tent", bufs=8) as tentp,
                tc.tile_pool(name="outp", bufs=3) as outp,
                tc.tile_pool(name="acc", bufs=12) as accp,
                tc.tile_pool(name="mtmpa", bufs=4) as mtmpp,
                tc.tile_pool(name="yg", bufs=3) as ygp,
                tc.tile_pool(name="ygps", bufs=2, space="PSUM") as ygpsp,
                tc.tile_pool(name="gps", bufs=1, space="PSUM") as gpsp,
            ):
                for blk in range(2):
                    r0 = blk * 128
                    xt = []
                    for ch in range(C):
                        t = xsp.tile([128, W], F32, tag="x")
                        nc.sync.dma_start(t[:], xs[ch, r0: r0 + 128, :])
                        xt.append(t)
                    # guide -> d in [0,7]
                    lum = dgp.tile([128, W], F32, tag="lum")
                    t2 = dgp.tile([128, W], F32, tag="lt")
                    nc.vector.tensor_scalar(lum[:], xt[0][:], 0.299, None,
                                            op0=Alu.mult)
                    nc.vector.tensor_scalar(t2[:], xt[1][:], 0.587, None,
                                            op0=Alu.mult)
                    nc.vector.tensor_tensor(lum[:], lum[:], t2[:], Alu.add)
                    nc.vector.tensor_scalar(t2[:], xt[2][:], 0.114, None,
                                            op0=Alu.mult)
                    nc.vector.tensor_tensor(lum[:], lum[:], t2[:], Alu.add)
                    nc.vector.tensor_scalar(lum[:], lum[:], 0.0, 1.0,
                                            op0=Alu.max, op1=Alu.min)
                    nc.vector.tensor_scalar(lum[:], lum[:], 7.0, None,
                                            op0=Alu.mult)
                    tents = []
                    for k in range(8):
                        # w_k = relu(1-|d-k|) = relu(min(d-(k-1), (k+1)-d))
                        u = tentp.tile([128, W], F32, tag="tent")
                        ub = dgp.tile([128, W], F32, tag="tb")
                        nc.vector.tensor_scalar(u[:], lum[:], float(k - 1),
                                                None, op0=Alu.subtract)
                        nc.vector.tensor_scalar(ub[:], lum[:], -1.0,
                                                float(k + 1), op0=Alu.mult,
                                                op1=Alu.add)
                        nc.vector.tensor_tensor(u[:], u[:], ub[:], Alu.min)
                        nc.vector.tensor_scalar(u[:], u[:], 0.0, None,
                                                op0=Alu.max)
                        tents.append(u)

                    accs = []
                    for cgrp in range(12):
                        # stage-Y: 8 channels (c=cgrp, k=0..7)
                        ygps = ygpsp.tile([WG, 8, 128], F32, tag="ygps")
                        for k in range(8):
                            mm(ygps[:, k, :], gridT[:, cgrp * 8 + k, :],
                               wyS[:, r0: r0 + 128], start=True, stop=True)
                        yg = ygp.tile([WG, 8, 128], F32, tag="yg")
                        nc.scalar.copy(yg[:, :, :], ygps[:, :, :])
                        acc = accp.tile([128, W], F32, tag="acc")
                        accs.append(acc)
                        for kp in range(4):
                            # two depth-planes share one PSUM tile: halves
                            # the PE->DVE handoff count
                            gp2 = gpsp.tile([128, 2, W], F32, tag="gps")
                            for kh in range(2):
                                k = 2 * kp + kh
                                lhsT = yg[:, k, :]
                                mm(gp2[:, kh, 0:512], lhsT, wxS[:, 0:512],
                                   start=True, stop=True)
                                mm(gp2[:, kh, 512:1024], lhsT,
                                   wxS[:, 512:1024], start=True, stop=True)
                            for kh in range(2):
                                k = 2 * kp + kh
                                if k == 0:
                                    nc.vector.tensor_tensor(
                                        acc[:], tents[0][:], gp2[:, 0, :],
                                        Alu.mult)
                                else:
                                    tm = mtmpp.tile([128, W], F32, tag="mt")
                                    nc.vector.tensor_tensor(
                                        tm[:], tents[k][:], gp2[:, kh, :],
                                        Alu.mult)
                                    nc.vector.tensor_tensor(
                                        acc[:], acc[:], tm[:], Alu.add)
                    # affine apply + clip
                    for i in range(3):
                        o = accs[4 * i + 3]
                        for jj in range(3):
                            tm = mtmpp.tile([128, W], F32, tag="at")
                            nc.vector.tensor_tensor(
                                tm[:], accs[4 * i + jj][:], xt[jj][:],
                                Alu.mult)
                            nc.vector.tensor_tensor(o[:], o[:], tm[:],
                                                    Alu.add)
                        nc.vector.tensor_scalar(o[:], o[:], 0.0, 1.0,
                                                op0=Alu.max, op1=Alu.min)
                        # quantize to uint16 (DVE convert rounds to nearest)
                        ot = outp.tile([128, W], mybir.dt.uint16, tag="out")
                        nc.vector.tensor_scalar(ot[:], o[:], 65535.0, None,
                                                op0=Alu.mult)
                        nc.sync.dma_start(ys[i, r0: r0 + 128, :], ot[:])
    nc.compile()
    return nc


def _host_prep(x, ws):
    x = np.ascontiguousarray(x, np.float32)
    t = 0.5 * (x[:, :, 1::4, :] + x[:, :, 2::4, :])
    xl = 0.5 * (t[:, :, :, 1::4] + t[:, :, :, 2::4])  # [B,3,256,256]

    bands = np.empty((N_CORES, GN, C, 14, 256), np.float32)
    for k in range(N_CORES):
        b, q = k // 4, k % 4
        for j in range(GN):
            g = GBASE[q] + j
            bands[k, j] = xl[b, :, 16 * g + 1: 16 * g + 15, :]

    if "const" in _CACHED:
        wyT, wxm = _CACHED["const"]
        cws, cbs = _repack_weights(ws)
        return x, bands, wyT, wxm, cws, cbs

    ysf = np.arange(H, dtype=np.float32) * np.float32((HG - 1) / (H - 1))
    y0 = np.floor(ysf).astype(np.int32)
    wy = (ysf - y0).astype(np.float32)
    wyT = np.zeros((N_CORES, GN, STRIP), np.float32)
    for k in range(N_CORES):
        q = k % 4
        rs = q * STRIP
        for r in range(STRIP):
            rr = rs + r
            j0 = y0[rr] - GBASE[q]
            j1 = min(y0[rr] + 1, HG - 1) - GBASE[q]
            wyT[k, j0, r] += np.float32(1.0) - wy[rr]
            wyT[k, j1, r] += wy[rr]

    xsf = np.arange(W, dtype=np.float32) * np.float32((WG - 1) / (W - 1))
    x0 = np.floor(xsf).astype(np.int32)
    wx = (xsf - x0).astype(np.float32)
    wxm = np.zeros((WG, W), np.float32)
    for ccol in range(W):
        x1 = min(x0[ccol] + 1, WG - 1)
        wxm[x0[ccol], ccol] += np.float32(1.0) - wx[ccol]
        wxm[x1, ccol] += wx[ccol]

    _CACHED["const"] = (wyT, wxm)
    cws, cbs = _repack_weights(ws)
    return x, bands, wyT, wxm, cws, cbs


def _repack_weights(ws):
    cws, cbs = [], []
    for i in range(6):
        wl = np.asarray(ws[2 * i], np.float32)  # [O, I, 3, 3]
        O, I = wl.shape[0], wl.shape[1]
        if i == 0:
            # [27, 1, O]: q = dy*9 + dx*3 + i
            g0 = wl.transpose(2, 3, 1, 0).reshape(27, 1, O)
            cws.append([np.ascontiguousarray(g0)])
        elif I <= 32:
            # [3I, 3, O]: per dy, q = dx*I + i
            g0 = np.stack([wl[:, :, dy, :].transpose(2, 1, 0).reshape(3 * I, O)
                           for dy in range(3)], axis=1)
            cws.append([np.ascontiguousarray(g0)])
        else:
            g0 = np.stack(
                [wl[:, :, dy, 0:2].transpose(2, 1, 0).reshape(2 * I, O)
                 for dy in range(3)], axis=1)
            g1 = np.stack([wl[:, :, dy, 2].transpose(1, 0)
                           for dy in range(3)], axis=1)
            cws.append([np.ascontiguousarray(g0), np.ascontiguousarray(g1)])
        cbs.append(np.ascontiguousarray(
            np.asarray(ws[2 * i + 1], np.float32).reshape(-1, 1)))
    return cws, cbs


def kernel(x, w1, b1, w2, b2, w3, b3, w4, b4, w5, b5, w6, b6):
    ws = (w1, b1, w2, b2, w3, b3, w4, b4, w5, b5, w6, b6)
    x, bands, wyT, wxm, cws, cbs = _host_prep(x, ws)
    if "nc" not in _CACHED:
        _CACHED["nc"] = _build_module()
    nc = _CACHED["nc"]

    blobs = _CACHED.setdefault(
        "blobs", [np.empty(NTOT, np.float32) for _ in range(N_CORES)])
    in_maps = []
    for k in range(N_CORES):
        b, s = k // 4, (k % 4) * STRIP
        blob = blobs[k]
        blob[_OFFS["xs"]: _OFFS["xs"] + C * STRIP * W] = \
            x[b, :, s: s + STRIP, :].reshape(-1)
        blob[_OFFS["bands"]: _OFFS["bands"] + bands[k].size] = \
            bands[k].reshape(-1)
        blob[_OFFS["wyT"]: _OFFS["wyT"] + wyT[k].size] = wyT[k].reshape(-1)
        blob[_OFFS["wxm"]: _OFFS["wxm"] + wxm.size] = wxm.reshape(-1)
        for i in range(6):
            for g, arr in enumerate(cws[i]):
                o = _OFFS[f"cw{i}g{g}"]
                blob[o: o + arr.size] = arr.reshape(-1)
            o = _OFFS[f"cb{i}"]
            blob[o: o + cbs[i].size] = cbs[i].reshape(-1)
        in_maps.append({"inp": blob})
    res = run_bass_kernel_spmd(nc, in_maps, core_ids=list(range(N_CORES)))
    _CACHED["last"] = res
    y = np.empty((B, C, H, W), np.float32)
    scale = np.float32(1.0 / 65535.0)
    for k in range(N_CORES):
        b, s = k // 4, (k % 4) * STRIP
        q = res.results[k]["ys"]
        np.multiply(q, scale, out=y[b, :, s: s + STRIP, :],
                    casting="unsafe")
    return y
